# revision 26
# baseline (speedup 1.0000x reference)
"""Trainium2 Bass kernel: separable box filter (radius 4) on (8,3,1024,1024) fp32.

v10: fp8 input, H-pass-first, d=6 direct / 21 scan balance, per-group DMAs.

 - Host casts x to fp8 e4m3 (L2 rel err 3.0e-3 vs the 2e-2 budget, measured
   on the true jax key-0 input).  Output fp16.
 - All 9 input DMAs (3 per slice) are issued on SP before anything else.
 - Per tile, the H (row) box pass is a banded matmul (lhsT[k,m]=1 iff
   m<=k<=m+8, zero-padded to 128 cols for FWL) over fp8: PSUM f32.
 - Tiles are processed in PAIRS sharing one [128,2048] PSUM tile (4 banks,
   ring 2): one ACT activation drains both tiles of a pair.
 - 21 "scan" tiles finish the W pass on the DVE (tensor_tensor_scan);
   6 "direct" tiles ({1,5} per slice) compute the full 9x9 on the PE via
   9 shifted band matmuls per half (measured: PE ~285 ns/matmul incl
   ldweights exposure, DVE ~2.3 us/scan — d=6 balances the two).
 - Each drain-group's output leaves in its own batched fp16 DMA as soon as
   the group finishes (short tail).
"""

import numpy as np

H = 1024
W = 1024
R = 4
D = 2 * R + 1
N_CORES = 8
SLICES_PER_CORE = 3
TILE = 120
N_TILES = 9
XW = 1036          # per-subtile pitch: 4 zeros | 1024 data | 8 pad
SXW = 9 * XW + 16  # slice input buffer width
YW = 1040          # drained fp16 rows: 9 zeros | 1024 data | 4 zeros | 3 slack
SW = 1028          # scan free size

DIRECT = (1, 5)
# (kind, tiles) per-slice emission order: scan groups first (feed the DVE),
# direct groups afterwards (PE tap stretches)
GROUPS = [("s", (0, 2)), ("s", (4, 6)), ("s", (3, 7)), ("s", (8,)),
          ("d", (1, 5))]

_COMPILED = {}


def _band_mid():
    k = np.arange(128)[:, None]
    m = np.arange(128)[None, :]
    return ((m <= k) & (k <= m + 2 * R) & (m < TILE)).astype(np.float32)


def _band_t0():
    k = np.arange(124)[:, None]
    m = np.arange(128)[None, :]
    return ((m - R <= k) & (k <= m + R) & (m < TILE)).astype(np.float32)


def _build():
    from concourse import bacc, mybir
    from concourse.tile import TileContext
    from concourse.ap import AP

    f8 = mybir.dt.float8e4
    f16 = mybir.dt.float16
    f32 = mybir.dt.float32
    nc = bacc.Bacc("TRN2", target_bir_lowering=False, debug=False,
                   num_devices=N_CORES)

    x = nc.dram_tensor("x", (SLICES_PER_CORE, H, W), f8,
                       kind="ExternalInput").ap()
    wp = nc.dram_tensor("wp", (128, 128), f8, kind="ExternalInput").ap()
    wp0 = nc.dram_tensor("wp0", (124, 128), f8, kind="ExternalInput").ap()
    out = nc.dram_tensor("out", (SLICES_PER_CORE, H, W), f16,
                         kind="ExternalOutput").ap()

    add = mybir.AluOpType.add
    sub = mybir.AluOpType.subtract
    act_copy = mybir.ActivationFunctionType.Copy

    xh = x.tensor
    oh = out.tensor

    def kp_of(t):
        return 124 if t == 0 else (68 if t == 8 else 128)

    def m_of(t):
        return 64 if t == 8 else TILE

    with TileContext(nc) as tc:
        with tc.tile_pool(name="wts", bufs=1) as wpool, \
             tc.tile_pool(name="xp", bufs=1) as xpool, \
             tc.tile_pool(name="yb", bufs=1) as ypool, \
             tc.tile_pool(name="st", bufs=3) as spool, \
             tc.tile_pool(name="ob", bufs=2) as opool, \
             tc.tile_pool(name="ps", bufs=2, space="PSUM") as pspool:

            # --- pad memsets first (no deps, run immediately) ---
            yb2s = []
            for i in range(4):
                yb2 = ypool.tile([TILE, 2, YW], f16, tag=f"yb{i}",
                                 name=f"yb{i}")
                yb2s.append(yb2)
                padl = AP(yb2[:, 0:1, 0:1].tensor, yb2[:, 0:1, 0:1].offset,
                          [[2 * YW, TILE], [YW, 2], [1, D]])
                nc.gpsimd.memset(padl, 0.0)
                padr = AP(yb2[:, 0:1, 0:1].tensor,
                          yb2[:, 0:1, 0:1].offset + D + W,
                          [[2 * YW, TILE], [YW, 2], [1, YW - D - W]])
                nc.gpsimd.memset(padr, 0.0)
            ygi = 0

            sxb = []
            for si in range(SLICES_PER_CORE):
                b = xpool.tile([128, SXW], f8, tag=f"sx{si}", name=f"sx{si}")
                sxb.append(b)
                nc.gpsimd.memset(b[:, 0:4], 0.0)
                gaps = AP(b[:, 0:1].tensor, b[:, 0:1].offset + 1028,
                          [[SXW, 128], [XW, 9], [1, 12]])
                nc.gpsimd.memset(gaps, 0.0)

            # --- all input DMAs up front on SP ---
            wp0_t = wpool.tile([124, 128], f8)
            nc.sync.dma_start(wp0_t[:], wp0[:])
            wp_t = wpool.tile([128, 128], f8)
            nc.sync.dma_start(wp_t[:], wp[:])
            for si in range(SLICES_PER_CORE):
                b = sxb[si]
                nc.sync.dma_start(b[0:124, 4:4 + W], x[si, 0:124, :])
                src_mid = AP(xh, si * H * W + (TILE - R) * W,
                             [[W, 128], [TILE * W, 7], [1, W]])
                dst_mid = AP(b[:, 0:1].tensor, b[:, 0:1].offset + XW + 4,
                             [[SXW, 128], [XW, 7], [1, W]])
                nc.sync.dma_start(dst_mid, src_mid)
                nc.sync.dma_start(b[0:68, 8 * XW + 4:8 * XW + 4 + W],
                                  x[si, 8 * TILE - R:H, :])

            def xv(b, t, a, bb, rows):
                return b[0:rows, XW * t + a:XW * t + bb]

            # --- phase 1: every slice's scan groups (DVE stream) ---
            for s in range(SLICES_PER_CORE):
                b = sxb[s]
                base = s * H * W
                for kind, tiles in GROUPS:
                    if kind != "s":
                        continue
                    nt = len(tiles)
                    ps = pspool.tile([128, 2048], f32)
                    for gi, t in enumerate(tiles):
                        kp = kp_of(t)
                        pbase = gi * 1024
                        lhs = wp0_t if t == 0 else wp_t
                        for hf in range(2):
                            w0 = 512 * hf
                            nc.tensor.matmul(
                                ps[:, pbase + w0:pbase + w0 + 512],
                                lhs[0:kp, :],
                                xv(b, t, 4 + w0, 4 + w0 + 512, kp),
                                start=True, stop=True)
                    yb2 = yb2s[ygi % 4]
                    ygi += 1
                    dst = AP(yb2[:, 0:1, 0:1].tensor,
                             yb2[:, 0:1, 0:1].offset + D,
                             [[2 * YW, TILE], [YW, nt], [1, W]])
                    nc.scalar.activation(dst, ps[0:TILE, 0:1024 * nt],
                                         act_copy)
                    st = spool.tile([TILE, 2, SW], f16, tag="st2",
                                    name="st2")
                    for gi, t in enumerate(tiles):
                        m = m_of(t)
                        nc.vector.tensor_tensor_scan(
                            st[0:m, gi, :], yb2[0:m, gi, D:D + SW],
                            yb2[0:m, gi, 0:SW], 0.0, add, sub)
                    stride = (tiles[1] - tiles[0]) * TILE * W if nt == 2 \
                        else TILE * W
                    rows = m_of(tiles[-1])
                    dsto = AP(oh, base + TILE * tiles[0] * W,
                              [[W, rows], [stride, nt], [1, W]])
                    nc.sync.dma_start(dsto, st[0:rows, 0:nt, R:R + W])

            # --- phase 2: direct tap stretches on the PE ---
            for s in range(SLICES_PER_CORE):
                b = sxb[s]
                base = s * H * W
                for kind, tiles in GROUPS:
                    if kind != "d":
                        continue
                    nt = len(tiles)
                    ps = pspool.tile([128, 2048], f32)
                    for gi, t in enumerate(tiles):
                        kp = kp_of(t)
                        pbase = gi * 1024
                        for hf in range(2):
                            w0 = 512 * hf
                            for j in range(D):
                                nc.tensor.matmul(
                                    ps[:, pbase + w0:pbase + w0 + 512],
                                    wp_t[0:kp, :],
                                    xv(b, t, w0 + j, w0 + j + 512, kp),
                                    start=(j == 0), stop=(j == D - 1))
                    ob = opool.tile([TILE, 2, W], f16, tag="ob", name="ob")
                    dst = AP(ob[:, 0:1, 0:1].tensor, ob[:, 0:1, 0:1].offset,
                             [[2 * W, TILE], [W, nt], [1, W]])
                    nc.scalar.activation(dst, ps[0:TILE, 0:1024 * nt],
                                         act_copy)
                    stride = (tiles[1] - tiles[0]) * TILE * W
                    dsto = AP(oh, base + TILE * tiles[0] * W,
                              [[W, TILE], [stride, nt], [1, W]])
                    nc.sync.dma_start(dsto, ob[0:TILE, 0:nt, :])

    nc.compile()
    return nc


def _get_nc():
    if "nc" not in _COMPILED:
        _COMPILED["nc"] = _build()
    return _COMPILED["nc"]


def _in_maps(x: np.ndarray):
    import ml_dtypes

    f8 = ml_dtypes.float8_e4m3fn
    xf = np.ascontiguousarray(np.asarray(x).astype(f8)).reshape(
        N_CORES * SLICES_PER_CORE, H, W)
    return [{
        "x": xf[c * SLICES_PER_CORE:(c + 1) * SLICES_PER_CORE],
        "wp": _band_mid().astype(f8),
        "wp0": _band_t0().astype(f8),
    } for c in range(N_CORES)]


def kernel(x: np.ndarray) -> np.ndarray:
    from concourse.bass_utils import run_bass_kernel_spmd

    nc = _get_nc()
    res = run_bass_kernel_spmd(nc, _in_maps(x), core_ids=list(range(N_CORES)))
    outs = [res.results[c]["out"] for c in range(N_CORES)]
    return np.concatenate(outs, axis=0).reshape(8, 3, H, W).astype(np.float32)


# revision 27
# speedup vs baseline: 1.2180x; 1.2180x over previous
"""Trainium2 Bass kernel: separable box filter (radius 4) on (8,3,1024,1024) fp32.

v6: DVE/PE hybrid, fp16 end-to-end (host casts f32<->f16; rel-err budget
2e-2 vs ~7e-4 worst-case fp16 error).

 - W pass (7 tiles/slice) on DVE: tensor_tensor_scan running 9-tap box sum
   (state is fp32 internally).  This is the kernel's hard floor: the scan
   runs at 2.08 ns/elem with no fast modes (measured).
 - Two tiles per slice (t=4,5) skip the scan: the PE computes their 9x9 box
   directly as 9 W-shifted accumulating band matmuls per 512-wide half,
   emitted as one contiguous stretch so the PE p-state ramps.
 - H pass: banded weights W[k, m] = 1 iff m <= k <= m+8.  Edge tiles use
   K-sliced matmuls instead of zeroed halo partitions (tile 0 loads rows
   0..123 unshifted with its own band wp0; tile 8 contracts over K=68), so
   input DMAs carry no waits and the schedule keeps them first.
 - ACT drains PSUM f32 -> SBUF fp16; scan-PSUM ring depth 3 keeps the tail
   matmul->ACT chain from serializing; direct-PSUM is a half-width ring so
   drains overlap the stretch.
"""

import numpy as np

H = 1024
W = 1024
R = 4
D = 2 * R + 1
N_CORES = 8
SLICES_PER_CORE = 3
TILE_OUT = 120
N_TILES = 9
P_W = D + W + R  # 9 left zeros + 1024 data + 4 right zeros
S_W = W + R

DIRECT = (4, 5)
SCAN_TILES = [0, 1, 2, 3, 6, 7, 8]

_COMPILED = {}


def _band_weights():
    """Standard band: lhsT[k, m] = 1 iff m <= k <= m+8 (tile rows start at
    global row 120t-4)."""
    k = np.arange(128)[:, None]
    m = np.arange(TILE_OUT)[None, :]
    return ((m <= k) & (k <= m + 2 * R)).astype(np.float16)


def _band_weights0():
    """Tile-0 band for unshifted load (partition p = global row p):
    lhsT[k, m] = 1 iff m-4 <= k <= m+4 (left truncation via k >= 0)."""
    k = np.arange(128)[:, None]
    m = np.arange(TILE_OUT)[None, :]
    return ((m - R <= k) & (k <= m + R) & (k < 124)).astype(np.float16)


def _build():
    from concourse import bacc, mybir
    from concourse.tile import TileContext

    f16 = mybir.dt.float16
    f32 = mybir.dt.float32
    nc = bacc.Bacc("TRN2", target_bir_lowering=False, debug=False,
                   num_devices=N_CORES)

    x = nc.dram_tensor("x", (SLICES_PER_CORE, H, W), f16,
                       kind="ExternalInput").ap()
    wp = nc.dram_tensor("wp", (128, TILE_OUT), f16, kind="ExternalInput").ap()
    wp0 = nc.dram_tensor("wp0", (128, TILE_OUT), f16,
                         kind="ExternalInput").ap()
    out = nc.dram_tensor("out", (SLICES_PER_CORE, H, W), f16,
                         kind="ExternalOutput").ap()

    add = mybir.AluOpType.add
    sub = mybir.AluOpType.subtract
    act_copy = mybir.ActivationFunctionType.Copy

    from concourse.ap import AP

    xh = x.tensor
    oh = out.tensor

    def src_windows(s, t0, nt):
        off = s * H * W + (TILE_OUT * t0 - R) * W
        return AP(xh, off, [[W, 128], [TILE_OUT * W, nt], [1, W]])

    def dst_rows(s, t0, nt):
        off = s * H * W + TILE_OUT * t0 * W
        return AP(oh, off, [[W, TILE_OUT], [TILE_OUT * W, nt], [1, W]])

    def in_dma(xc, s, t):
        if t == 0:
            # unshifted: partition p = global row p, rows 0..123
            nc.sync.dma_start(xc[0:124, D:D + W], x[s, 0:124, :])
        elif t == 8:
            nc.sync.dma_start(xc[0:68, D:D + W], x[s, 8 * TILE_OUT - R:H, :])
        else:
            nc.sync.dma_start(xc[:, D:D + W], src_windows(s, t, 1)[:, 0, :])

    def pad_cols(xc):
        nc.gpsimd.memset(xc[:, 0:D], 0.0)
        nc.gpsimd.memset(xc[:, D + W:P_W], 0.0)

    def band_for(t):
        if t == 0:
            return 124  # wp0, K=124
        if t == 8:
            return 68  # wp, K=68
        return 128

    with TileContext(nc) as tc:
        with tc.tile_pool(name="wts", bufs=1) as wpool, \
             tc.tile_pool(name="xp", bufs=1) as xpool, \
             tc.tile_pool(name="sc", bufs=10) as spool, \
             tc.tile_pool(name="outp", bufs=8) as opool, \
             tc.tile_pool(name="ps", bufs=3, space="PSUM") as pspool, \
             tc.tile_pool(name="psd", bufs=2, space="PSUM") as dpool:
            xbufs = []
            for t in range(N_TILES):
                xb = xpool.tile([128, P_W], f16, tag=f"xc{t}")
                xbufs.append(xb)

            wp_t = wpool.tile([128, TILE_OUT], f16)
            nc.scalar.dma_start(wp_t[:], wp[:])
            wp0_t = wpool.tile([128, TILE_OUT], f16)
            nc.scalar.dma_start(wp0_t[:], wp0[:])

            for s in range(SLICES_PER_CORE):
                # scan tile 0's input first: wait-free DMA, shortest fill
                in_dma(xbufs[0], s, 0)
                if s == 0:
                    pad_cols(xbufs[0])

                # ---- direct (PE) pair: 9 shifted band matmuls per half ----
                for t in DIRECT:
                    in_dma(xbufs[t], s, t)
                    if s == 0:
                        pad_cols(xbufs[t])
                ocd = opool.tile([TILE_OUT, 2, W], f16, tag="ocd")
                for di, t in enumerate(DIRECT):
                    xc = xbufs[t]
                    for hf in range(2):
                        w0 = 512 * hf
                        psd = dpool.tile([TILE_OUT, 512], f32)
                        for j in range(D):
                            nc.tensor.matmul(
                                psd[:], wp_t[:],
                                xc[:, 5 + w0 + j:5 + w0 + j + 512],
                                start=(j == 0), stop=(j == D - 1))
                        nc.scalar.activation(ocd[:, di, w0:w0 + 512],
                                             psd[:], act_copy)
                nc.scalar.dma_start(dst_rows(s, DIRECT[0], 2), ocd[:, 0:2, :])

                # ---- scan tiles on DVE ----
                # last slice: interleave so the final ACT drains spread out
                # (tail shrink); pairs stay adjacent for the output DMA.
                order = [0, 1, 6, 7, 2, 3, 8] if s == 2 else SCAN_TILES
                for idx, t in enumerate(order):
                    xc = xbufs[t]
                    if t != 0:
                        in_dma(xc, s, t)
                        if s == 0:
                            pad_cols(xc)

                    kp = band_for(t)
                    if idx % 2 == 0:
                        oc = opool.tile([TILE_OUT, 2, W], f16, tag="oc")
                    oi = idx % 2
                    m = min(TILE_OUT, H - TILE_OUT * t)
                    st = spool.tile([128, S_W], f16)
                    nc.vector.tensor_tensor_scan(
                        st[0:kp, :], xc[0:kp, D:P_W], xc[0:kp, 0:S_W],
                        0.0, add, sub)
                    lhs = wp0_t if t == 0 else wp_t
                    ps = pspool.tile([TILE_OUT, 2 * 512], f32)
                    for hf in range(2):
                        w0 = 512 * hf
                        nc.tensor.matmul(ps[0:m, w0:w0 + 512],
                                         lhs[0:kp, 0:m],
                                         st[0:kp, w0 + R:w0 + R + 512],
                                         start=True, stop=True)
                    if s == 2 and t == 8:
                        # final tile: drain PSUM on the DVE (idle after its
                        # last scan) instead of the still-busy ACT queue
                        nc.vector.tensor_copy(oc[0:m, oi, :], ps[0:m, :])
                    else:
                        nc.scalar.activation(oc[0:m, oi, :], ps[0:m, :],
                                             act_copy)
                    if t == 8:
                        nc.scalar.dma_start(out[s, 8 * TILE_OUT:H, :],
                                            oc[0:64, 0, :])
                    elif oi == 1:
                        nc.scalar.dma_start(dst_rows(s, t - 1, 2),
                                            oc[:, 0:2, :])

    nc.compile()
    return nc


def _get_nc():
    if "nc" not in _COMPILED:
        _COMPILED["nc"] = _build()
    return _COMPILED["nc"]


def _in_maps(x: np.ndarray):
    xf = np.ascontiguousarray(np.asarray(x, dtype=np.float16)).reshape(
        N_CORES * SLICES_PER_CORE, H, W)
    wp_np = _band_weights()
    wp0_np = _band_weights0()
    return [{
        "x": xf[c * SLICES_PER_CORE:(c + 1) * SLICES_PER_CORE],
        "wp": wp_np,
        "wp0": wp0_np,
    } for c in range(N_CORES)]


def kernel(x: np.ndarray) -> np.ndarray:
    from concourse.bass_utils import run_bass_kernel_spmd

    nc = _get_nc()
    res = run_bass_kernel_spmd(nc, _in_maps(x), core_ids=list(range(N_CORES)))
    outs = [res.results[c]["out"] for c in range(N_CORES)]
    return np.concatenate(outs, axis=0).reshape(8, 3, H, W).astype(np.float32)



# revision 28
# speedup vs baseline: 1.2736x; 1.0456x over previous
"""Trainium2 Bass kernel: separable box filter (radius 4) on (8,3,1024,1024) fp32.

v12: baseline scan-first structure + fp8 input + batched DMA.

 - W pass (7 tiles/slice) on DVE: tensor_tensor_scan running 9-tap box sum
   directly on the fp8 input (fp32 state, fp16 out).
 - Two tiles per slice (t=4,5) skip the scan: the PE computes their 9x9 box
   directly as 9 W-shifted accumulating band matmuls per 512-wide half.
 - H pass: banded weights W[k, m] = 1 iff m <= k <= m+8 over the scan
   output (fp16); tile 0 unshifted with its own band wp0; tile 8 K=68.
 - fp8 input halves the input HBM traffic (L2 rel err 3.0e-3 vs the 2e-2
   budget on the true key-0 input); inputs arrive as 3 batched DMAs per
   slice into one padded slice buffer, issued up front on SP.
 - Output DMAs ride SP too, keeping the ACT queue for PSUM drains only.
"""

import numpy as np

H = 1024
W = 1024
R = 4
D = 2 * R + 1
N_CORES = 8
SLICES_PER_CORE = 3
TILE_OUT = 120
N_TILES = 9
S_W = W + R            # 1028 scan free size
XW = 1040              # subtile pitch: 9 left zeros | 1024 | 4 zeros | 3 pad
SXW = 9 * XW + 16

DIRECT = (4, 5)
SCAN_TILES = [0, 1, 2, 3, 6, 7, 8]

_COMPILED = {}


def _band_weights():
    k = np.arange(128)[:, None]
    m = np.arange(TILE_OUT)[None, :]
    return ((m <= k) & (k <= m + 2 * R)).astype(np.float16)


def _band_weights0():
    k = np.arange(128)[:, None]
    m = np.arange(TILE_OUT)[None, :]
    return ((m - R <= k) & (k <= m + R) & (k < 124)).astype(np.float16)


def _build():
    from concourse import bacc, mybir
    from concourse.tile import TileContext
    from concourse.ap import AP

    f8 = mybir.dt.float8e4
    f16 = mybir.dt.float16
    f32 = mybir.dt.float32
    nc = bacc.Bacc("TRN2", target_bir_lowering=False, debug=False,
                   num_devices=N_CORES)

    x = nc.dram_tensor("x", (SLICES_PER_CORE, H, W), f8,
                       kind="ExternalInput").ap()
    wp = nc.dram_tensor("wp", (128, TILE_OUT), f16, kind="ExternalInput").ap()
    wp0 = nc.dram_tensor("wp0", (128, TILE_OUT), f16,
                         kind="ExternalInput").ap()
    out = nc.dram_tensor("out", (SLICES_PER_CORE, H, W), f16,
                         kind="ExternalOutput").ap()

    add = mybir.AluOpType.add
    sub = mybir.AluOpType.subtract
    act_copy = mybir.ActivationFunctionType.Copy

    xh = x.tensor
    oh = out.tensor

    def dst_rows(s, t0, nt):
        off = s * H * W + TILE_OUT * t0 * W
        return AP(oh, off, [[W, TILE_OUT], [TILE_OUT * W, nt], [1, W]])

    def band_for(t):
        if t == 0:
            return 124
        if t == 8:
            return 68
        return 128

    with TileContext(nc) as tc:
        with tc.tile_pool(name="wts", bufs=1) as wpool, \
             tc.tile_pool(name="xp", bufs=1) as xpool, \
             tc.tile_pool(name="sc", bufs=10) as spool, \
             tc.tile_pool(name="outp", bufs=8) as opool, \
             tc.tile_pool(name="ps", bufs=3, space="PSUM") as pspool, \
             tc.tile_pool(name="psd", bufs=2, space="PSUM") as dpool:

            # pad memsets first (no deps)
            sxb = []
            for si in range(SLICES_PER_CORE):
                b = xpool.tile([128, SXW], f8, tag=f"sx{si}", name=f"sx{si}")
                sxb.append(b)
                nc.gpsimd.memset(b[:, 0:D], 0.0)
                gaps = AP(b[:, 0:1].tensor, b[:, 0:1].offset + 1033,
                          [[SXW, 128], [XW, 9], [1, 16]])
                nc.gpsimd.memset(gaps, 0.0)

            # weights + all input DMAs up front on SP
            wp_t = wpool.tile([128, TILE_OUT], f16)
            nc.sync.dma_start(wp_t[:], wp[:])
            wp0_t = wpool.tile([128, TILE_OUT], f16)
            nc.sync.dma_start(wp0_t[:], wp0[:])
            for si in range(SLICES_PER_CORE):
                b = sxb[si]
                nc.sync.dma_start(b[0:124, D:D + W], x[si, 0:124, :])
                src_mid = AP(xh, si * H * W + (TILE_OUT - R) * W,
                             [[W, 128], [TILE_OUT * W, 7], [1, W]])
                dst_mid = AP(b[:, 0:1].tensor, b[:, 0:1].offset + XW + D,
                             [[SXW, 128], [XW, 7], [1, W]])
                nc.sync.dma_start(dst_mid, src_mid)
                nc.sync.dma_start(b[0:68, 8 * XW + D:8 * XW + D + W],
                                  x[si, 8 * TILE_OUT - R:H, :])

            def xc(s, t, a, bb, rows=128):
                return sxb[s][0:rows, XW * t + a:XW * t + bb]

            for s in range(SLICES_PER_CORE):
                # ---- direct (PE) pair ----
                ocd = opool.tile([TILE_OUT, 2, W], f16, tag="ocd")
                for di, t in enumerate(DIRECT):
                    for hf in range(2):
                        w0 = 512 * hf
                        psd = dpool.tile([TILE_OUT, 512], f32)
                        for j in range(D):
                            nc.tensor.matmul(
                                psd[:], wp_t[:],
                                xc(s, t, 5 + w0 + j, 5 + w0 + j + 512),
                                start=(j == 0), stop=(j == D - 1))
                        nc.scalar.activation(ocd[:, di, w0:w0 + 512],
                                             psd[:], act_copy)
                nc.sync.dma_start(dst_rows(s, DIRECT[0], 2), ocd[:, 0:2, :])

                # ---- scan tiles on DVE (scan reads raw fp8) ----
                order = [0, 1, 6, 7, 2, 3, 8] if s == 2 else SCAN_TILES
                for idx, t in enumerate(order):
                    kp = band_for(t)
                    if idx % 2 == 0:
                        oc = opool.tile([TILE_OUT, 2, W], f16, tag="oc")
                    oi = idx % 2
                    m = min(TILE_OUT, H - TILE_OUT * t)
                    st = spool.tile([128, S_W], f16)
                    nc.vector.tensor_tensor_scan(
                        st[0:kp, :], xc(s, t, D, D + S_W, kp),
                        xc(s, t, 0, S_W, kp), 0.0, add, sub)
                    lhs = wp0_t if t == 0 else wp_t
                    ps = pspool.tile([TILE_OUT, 2 * 512], f32)
                    for hf in range(2):
                        w0 = 512 * hf
                        nc.tensor.matmul(ps[0:m, w0:w0 + 512],
                                         lhs[0:kp, 0:m],
                                         st[0:kp, w0 + R:w0 + R + 512],
                                         start=True, stop=True)
                    if s == 2 and t == 8:
                        nc.vector.tensor_copy(oc[0:m, oi, :], ps[0:m, :])
                    else:
                        nc.scalar.activation(oc[0:m, oi, :], ps[0:m, :],
                                             act_copy)
                    if t == 8:
                        nc.sync.dma_start(out[s, 8 * TILE_OUT:H, :],
                                          oc[0:64, 0, :])
                    elif oi == 1:
                        nc.sync.dma_start(dst_rows(s, t - 1, 2),
                                          oc[:, 0:2, :])

    nc.compile()
    return nc


def _get_nc():
    if "nc" not in _COMPILED:
        _COMPILED["nc"] = _build()
    return _COMPILED["nc"]


def _in_maps(x: np.ndarray):
    import ml_dtypes

    f8 = ml_dtypes.float8_e4m3fn
    xf = np.ascontiguousarray(np.asarray(x).astype(f8)).reshape(
        N_CORES * SLICES_PER_CORE, H, W)
    return [{
        "x": xf[c * SLICES_PER_CORE:(c + 1) * SLICES_PER_CORE],
        "wp": _band_weights(),
        "wp0": _band_weights0(),
    } for c in range(N_CORES)]


def kernel(x: np.ndarray) -> np.ndarray:
    from concourse.bass_utils import run_bass_kernel_spmd

    nc = _get_nc()
    res = run_bass_kernel_spmd(nc, _in_maps(x), core_ids=list(range(N_CORES)))
    outs = [res.results[c]["out"] for c in range(N_CORES)]
    return np.concatenate(outs, axis=0).reshape(8, 3, H, W).astype(np.float32)


# revision 29
# speedup vs baseline: 1.2800x; 1.0050x over previous
"""Trainium2 Bass kernel: separable box filter (radius 4) on (8,3,1024,1024) fp32.

v15: scan-first fp8, weights on scalar, all drains on ACT + fp8 input + batched DMA.

 - W pass (7 tiles/slice) on DVE: tensor_tensor_scan running 9-tap box sum
   directly on the fp8 input (fp32 state, fp16 out).
 - Two tiles per slice (t=4,5) skip the scan: the PE computes their 9x9 box
   directly as 9 W-shifted accumulating band matmuls per 512-wide half.
 - H pass: banded weights W[k, m] = 1 iff m <= k <= m+8 over the scan
   output (fp16); tile 0 unshifted with its own band wp0; tile 8 K=68.
 - fp8 input halves the input HBM traffic (L2 rel err 3.0e-3 vs the 2e-2
   budget on the true key-0 input); inputs arrive as 3 batched DMAs per
   slice into one padded slice buffer, issued up front on SP.
 - Output DMAs ride SP too, keeping the ACT queue for PSUM drains only.
"""

import numpy as np

H = 1024
W = 1024
R = 4
D = 2 * R + 1
N_CORES = 8
SLICES_PER_CORE = 3
TILE_OUT = 120
N_TILES = 9
S_W = W + R            # 1028 scan free size
XW = 1040              # subtile pitch: 9 left zeros | 1024 | 4 zeros | 3 pad
SXW = 9 * XW + 16

DIRECT = (4, 5)
SCAN_TILES = [0, 1, 2, 3, 6, 7, 8]

_COMPILED = {}


def _band_weights():
    k = np.arange(128)[:, None]
    m = np.arange(TILE_OUT)[None, :]
    return ((m <= k) & (k <= m + 2 * R)).astype(np.float16)


def _band_weights0():
    k = np.arange(128)[:, None]
    m = np.arange(TILE_OUT)[None, :]
    return ((m - R <= k) & (k <= m + R) & (k < 124)).astype(np.float16)


def _build():
    from concourse import bacc, mybir
    from concourse.tile import TileContext
    from concourse.ap import AP

    f8 = mybir.dt.float8e4
    f16 = mybir.dt.float16
    f32 = mybir.dt.float32
    nc = bacc.Bacc("TRN2", target_bir_lowering=False, debug=False,
                   num_devices=N_CORES)

    x = nc.dram_tensor("x", (SLICES_PER_CORE, H, W), f8,
                       kind="ExternalInput").ap()
    wp = nc.dram_tensor("wp", (128, TILE_OUT), f16, kind="ExternalInput").ap()
    wp0 = nc.dram_tensor("wp0", (128, TILE_OUT), f16,
                         kind="ExternalInput").ap()
    out = nc.dram_tensor("out", (SLICES_PER_CORE, H, W), f16,
                         kind="ExternalOutput").ap()

    add = mybir.AluOpType.add
    sub = mybir.AluOpType.subtract
    act_copy = mybir.ActivationFunctionType.Copy

    xh = x.tensor
    oh = out.tensor

    def dst_rows(s, t0, nt):
        off = s * H * W + TILE_OUT * t0 * W
        return AP(oh, off, [[W, TILE_OUT], [TILE_OUT * W, nt], [1, W]])

    def band_for(t):
        if t == 0:
            return 124
        if t == 8:
            return 68
        return 128

    with TileContext(nc) as tc:
        with tc.tile_pool(name="wts", bufs=1) as wpool, \
             tc.tile_pool(name="xp", bufs=1) as xpool, \
             tc.tile_pool(name="sc", bufs=10) as spool, \
             tc.tile_pool(name="outp", bufs=8) as opool, \
             tc.tile_pool(name="ps", bufs=3, space="PSUM") as pspool, \
             tc.tile_pool(name="psd", bufs=2, space="PSUM") as dpool:

            # pad memsets first (no deps)
            sxb = []
            for si in range(SLICES_PER_CORE):
                b = xpool.tile([128, SXW], f8, tag=f"sx{si}", name=f"sx{si}")
                sxb.append(b)
                nc.gpsimd.memset(b[:, 0:D], 0.0)
                gaps = AP(b[:, 0:1].tensor, b[:, 0:1].offset + 1033,
                          [[SXW, 128], [XW, 9], [1, 16]])
                nc.gpsimd.memset(gaps, 0.0)

            # weights ride the (idle) scalar queue so SP starts the
            # input stream immediately
            wp_t = wpool.tile([128, TILE_OUT], f16)
            nc.scalar.dma_start(wp_t[:], wp[:])
            wp0_t = wpool.tile([128, TILE_OUT], f16)
            nc.scalar.dma_start(wp0_t[:], wp0[:])
            for si in range(SLICES_PER_CORE):
                b = sxb[si]
                nc.sync.dma_start(b[0:124, D:D + W], x[si, 0:124, :])
                src_mid = AP(xh, si * H * W + (TILE_OUT - R) * W,
                             [[W, 128], [TILE_OUT * W, 7], [1, W]])
                dst_mid = AP(b[:, 0:1].tensor, b[:, 0:1].offset + XW + D,
                             [[SXW, 128], [XW, 7], [1, W]])
                nc.sync.dma_start(dst_mid, src_mid)
                nc.sync.dma_start(b[0:68, 8 * XW + D:8 * XW + D + W],
                                  x[si, 8 * TILE_OUT - R:H, :])

            def xc(s, t, a, bb, rows=128):
                return sxb[s][0:rows, XW * t + a:XW * t + bb]

            for s in range(SLICES_PER_CORE):
                # ---- direct (PE) pair ----
                ocd = opool.tile([TILE_OUT, 2, W], f16, tag="ocd")
                for di, t in enumerate(DIRECT):
                    for hf in range(2):
                        w0 = 512 * hf
                        psd = dpool.tile([TILE_OUT, 512], f32)
                        for j in range(D):
                            nc.tensor.matmul(
                                psd[:], wp_t[:],
                                xc(s, t, 5 + w0 + j, 5 + w0 + j + 512),
                                start=(j == 0), stop=(j == D - 1))
                        nc.scalar.activation(ocd[:, di, w0:w0 + 512],
                                             psd[:], act_copy)
                nc.sync.dma_start(dst_rows(s, DIRECT[0], 2), ocd[:, 0:2, :])

                # ---- scan tiles on DVE (scan reads raw fp8) ----
                order = [0, 1, 6, 7, 2, 3, 8] if s == 2 else SCAN_TILES
                for idx, t in enumerate(order):
                    kp = band_for(t)
                    if idx % 2 == 0:
                        oc = opool.tile([TILE_OUT, 2, W], f16, tag="oc")
                    oi = idx % 2
                    m = min(TILE_OUT, H - TILE_OUT * t)
                    st = spool.tile([128, S_W], f16)
                    nc.vector.tensor_tensor_scan(
                        st[0:kp, :], xc(s, t, D, D + S_W, kp),
                        xc(s, t, 0, S_W, kp), 0.0, add, sub)
                    lhs = wp0_t if t == 0 else wp_t
                    ps = pspool.tile([TILE_OUT, 2 * 512], f32)
                    for hf in range(2):
                        w0 = 512 * hf
                        nc.tensor.matmul(ps[0:m, w0:w0 + 512],
                                         lhs[0:kp, 0:m],
                                         st[0:kp, w0 + R:w0 + R + 512],
                                         start=True, stop=True)
                    nc.scalar.activation(oc[0:m, oi, :], ps[0:m, :],
                                         act_copy)
                    if t == 8:
                        nc.sync.dma_start(out[s, 8 * TILE_OUT:H, :],
                                          oc[0:64, 0, :])
                    elif oi == 1:
                        nc.sync.dma_start(dst_rows(s, t - 1, 2),
                                          oc[:, 0:2, :])

    nc.compile()
    return nc


def _get_nc():
    if "nc" not in _COMPILED:
        _COMPILED["nc"] = _build()
    return _COMPILED["nc"]


def _in_maps(x: np.ndarray):
    import ml_dtypes

    f8 = ml_dtypes.float8_e4m3fn
    xf = np.ascontiguousarray(np.asarray(x).astype(f8)).reshape(
        N_CORES * SLICES_PER_CORE, H, W)
    return [{
        "x": xf[c * SLICES_PER_CORE:(c + 1) * SLICES_PER_CORE],
        "wp": _band_weights(),
        "wp0": _band_weights0(),
    } for c in range(N_CORES)]


def kernel(x: np.ndarray) -> np.ndarray:
    from concourse.bass_utils import run_bass_kernel_spmd

    nc = _get_nc()
    res = run_bass_kernel_spmd(nc, _in_maps(x), core_ids=list(range(N_CORES)))
    outs = [res.results[c]["out"] for c in range(N_CORES)]
    return np.concatenate(outs, axis=0).reshape(8, 3, H, W).astype(np.float32)
